# revision 1
# baseline (speedup 1.0000x reference)
"""Trainium2 Bass kernel for an LSTM caption decoder (DecoderRNN).

Math (fp32 reference):
  x_seq = [features; embedding[captions[:, :-1]]]      # [B, T, E]
  xg    = x_seq @ W + b                                # [T*B, 4H] (t-major)
  per step t: gates = xg_t + h @ U ; i,f,o=sigmoid, g=tanh
              c = f*c + i*g ; h = o*tanh(c)
  logits = hs @ linear_w.T + linear_b                  # [B, T, V]

B=64, T=64, E=512, H=1024, V=32000.

Distribution over 8 NeuronCores: the serial recurrence (and the xg GEMM
feeding it) is replicated on every core — its PE cost is K*N-bound and
independent of batch, so batch-splitting buys nothing and collectives
would cost more than the redundant compute. The vocab projection (84% of
total FLOPs) is sharded column-wise: core c computes logits[:, :, c*4000:
(c+1)*4000]. No collectives; the host concatenates the vocab slices.

All matmuls run in float32r (TF32: full PE rate, fp32 accumulate).
Weights/activations feeding matmuls are pre-rounded to the TF32 grid.
"""

from contextlib import ExitStack

import numpy as np

import concourse.bass as bass
import concourse.mybir as mybir
import concourse.tile as tile
from concourse import bacc, bass_utils
from concourse.bass import ds

F32 = mybir.dt.float32
F32R = mybir.dt.float32r
ACTF = mybir.ActivationFunctionType
import os as _os
IMM = _os.environ.get("KIMM", "1") == "1"

B, T, E, H, V = 64, 64, 512, 1024, 32000
NCORES = 8
VS = V // NCORES          # vocab slice per core (4000)
G4 = 4 * H                # 4096
TOK = T * B               # 4096 tokens, t-major (row = t*B + b)
EC = E // 128             # 4  E chunks
HC = H // 128             # 8  H chunks
MC = TOK // 128           # 32 token chunks (2 timesteps each)
NV = 8                    # vocab N-chunks per core
VN = VS // NV             # 500 columns per vocab N-chunk


def _round_tf32(x: np.ndarray) -> np.ndarray:
    """Round fp32 to the TF32 grid (RNE to 10 mantissa bits)."""
    bits = np.ascontiguousarray(x, dtype=np.float32).view(np.uint32)
    r = bits + np.uint32(0xFFF) + ((bits >> np.uint32(13)) & np.uint32(1))
    return (r & np.uint32(0xFFFFE000)).view(np.float32)


def _emit(ctx: ExitStack, tc: tile.TileContext, io: dict, phases="abc"):
    nc = tc.nc
    xT_d, w_d, u_d, bg_d, ident_d, wt_d, bl_d = (
        io["xT"], io["w"], io["u"], io["bg"], io["ident"], io["wt"], io["bl"])
    xg_d, hsT_d, out_d = io["xg_scratch"], io["hsT_scratch"], io["logits"]
    reps = io.get("reps", 1)

    if "a" in phases:
        _phase_a(tc, nc, xT_d, w_d, bg_d, xg_d, repeat=reps)
    if "b" in phases:
        _phase_b(tc, nc, u_d, ident_d, xg_d, hsT_d, repeat=reps)
    if "c" in phases:
        _phase_c(tc, nc, wt_d, bl_d, hsT_d, out_d, repeat=reps)


def _rep_loop(tc, nc, pool, repeat):
    """Repeat-loop context for timing (reps input) or None for repeat=1."""
    if isinstance(repeat, bass.AP):
        rt = pool.tile([1, 1], mybir.dt.int32, tag="reps", name="rt")
        nc.sync.dma_start(rt[:], repeat[:, :])
        with tc.tile_critical():
            tmp = nc.alloc_registers(f"reps_regs_{nc.next_id()}")
            nc.regs_load(tmp, rt[0:1, 0:1])
            n_reps = nc.snap(tmp, donate=True, min_val=1, max_val=1024)
        return tc.For_i(0, n_reps, 1)
    return tc.For_i(0, repeat, 1) if repeat > 1 else None


def _phase_a(tc, nc, xT_d, w_d, bg_d, xg_d, repeat=1):
    # ---------------- Phase A: xg = x @ W + b  ->  DRAM [TOK, 4H] --------
    with tc.tile_pool(name="a_sb", bufs=1) as a_sb, \
         tc.tile_pool(name="a_out", bufs=3) as a_out, \
         tc.tile_pool(name="a_ps", bufs=2, space="PSUM") as a_ps:
        xT_sb = [a_sb.tile([128, TOK], F32R, tag=f"xT{k}", name=f"xT{k}") for k in range(EC)]
        w_sb = [a_sb.tile([128, G4], F32R, tag=f"w{k}", name=f"wsb{k}") for k in range(EC)]
        bg_sb = a_sb.tile([128, G4], F32, tag="bg")
        for k in range(EC):
            nc.sync.dma_start(xT_sb[k][:], xT_d[k * 128:(k + 1) * 128, :])
            nc.sync.dma_start(w_sb[k][:], w_d[k * 128:(k + 1) * 128, :])
        nc.sync.dma_start(bg_sb[:], bg_d[:, :])

        rep_cm = _rep_loop(tc, nc, a_sb, repeat)
        if rep_cm is not None:
            rep_cm.__enter__()
        for m in range(MC):
            for half in range(2):
                ps = a_ps.tile([128, 2048], F32)
                for n in range(4):
                    col = half * 2048 + n * 512
                    for k in range(EC):
                        nc.tensor.matmul(
                            ps[:, n * 512:(n + 1) * 512],
                            xT_sb[k][:, m * 128:(m + 1) * 128],
                            w_sb[k][:, col:col + 512],
                            start=(k == 0), stop=(k == EC - 1))
                o_sb = a_out.tile([128, 2048], F32R)
                nc.vector.tensor_add(
                    o_sb[:], ps[:], bg_sb[:, half * 2048:(half + 1) * 2048])
                nc.sync.dma_start(
                    xg_d[m * 128:(m + 1) * 128, half * 2048:(half + 1) * 2048],
                    o_sb[:])
        if rep_cm is not None:
            rep_cm.__exit__(None, None, None)

def _phase_b(tc, nc, u_d, ident_d, xg_d, hsT_d, repeat=1):
    # ---------------- Phase B: the recurrence ---------------------------
    # gate column ranges in xg/U: i=[0,H), f=[H,2H), g=[2H,3H), o=[3H,4H)
    # PE order i, g, f, o so the c-chain (needs i,g,f) starts earliest.
    # xg_t is injected into PSUM via an identity matmul (keeps the DVE off
    # the critical path); each gate gets its own 2-bank PSUM tile.
    with tc.tile_pool(name="b_sb", bufs=1) as b_sb, \
         tc.tile_pool(name="b_xg", bufs=2) as b_xg, \
         tc.tile_pool(name="b_gate", bufs=5) as b_gate, \
         tc.tile_pool(name="b_tmp", bufs=3) as b_tmp, \
         tc.tile_pool(name="b_hT", bufs=24) as b_hT, \
         tc.tile_pool(name="b_ps", bufs=8, space="PSUM") as b_ps:
        u_sb = [b_sb.tile([128, G4], F32R, tag=f"u{k}", name=f"usb{k}") for k in range(HC)]
        for k in range(HC):
            nc.sync.dma_start(u_sb[k][:], u_d[k * 128:(k + 1) * 128, :])
        ident = b_sb.tile([64, 64], F32R, tag="ident")
        nc.sync.dma_start(ident[:], ident_d[:, :])
        c_st = b_sb.tile([64, H], F32, tag="c")     # persistent cell state
        h_st = b_sb.tile([64, H], F32, tag="h")     # persistent hidden (pre-T)

        rep_cm = _rep_loop(tc, nc, b_sb, repeat)
        if rep_cm is not None:
            rep_cm.__enter__()
        hT_prev = None
        for t in range(T):
            # xg_t [64, 4H] in one contiguous DMA (f32r, bias folded in)
            xg_t = b_xg.tile([64, G4], F32R, tag="xg")
            nc.sync.dma_start(xg_t[:], xg_d[t * 64:(t + 1) * 64, :])
            xg_g = [xg_t[:, g * H:(g + 1) * H] for g in range(4)]

            # One 1-bank PSUM tile per (gate, half). The I-MMs (psum = xg)
            # depend only on the xg DMA, so the PE can run them during the
            # previous step's elementwise tail. Emit them in pairs right
            # before their gate's U-MMs to avoid PE head-of-line blocking
            # on late PSUM slot recycling.
            GORDER = (0, 2, 1, 3)          # i, g, f, o
            ps_gh = {}

            def imm(g):
                for n2 in range(2):
                    ps = b_ps.tile([64, 512], F32, tag="ps",
                                   name=f"ps{g}_{n2}_{t}")
                    nc.tensor.matmul(
                        ps[:], ident[:], xg_g[g][:, n2 * 512:(n2 + 1) * 512],
                        start=True, stop=(t == 0))
                    ps_gh[(g, n2)] = ps

            def umms(g):
                for n2 in range(2):
                    ucol = g * H + n2 * 512
                    for k in range(HC):
                        nc.tensor.matmul(
                            ps_gh[(g, n2)][:], hT_prev[k][:],
                            u_sb[k][:, ucol:ucol + 512],
                            start=False, stop=(k == HC - 1))

            if t == 0:
                for g in GORDER:
                    imm(g)
            else:
                imm(0), imm(2)
                umms(0), umms(2)
                imm(1), imm(3)
                umms(1), umms(3)

            def act(g):
                a = b_gate.tile([64, H], F32, tag="gate", name=f"gate{g}_{t}")
                func = ACTF.Tanh if g == 2 else ACTF.Sigmoid
                for n2 in range(2):
                    nc.scalar.activation(a[:, n2 * 512:(n2 + 1) * 512],
                                         ps_gh[(g, n2)][:], func)
                return a

            # elementwise chain in 512-halves: i,g first, then f, then o
            i_t = act(0)
            g_t = act(2)
            ig = b_tmp.tile([64, H], F32, tag="tmp", name=f"ig{t}")
            f_t = act(1)
            tc_t = b_tmp.tile([64, H], F32, tag="tmp", name=f"tc{t}")
            o_t = act(3)
            hh_done = []
            for hh in range(2):
                sl = slice(hh * 512, (hh + 1) * 512)
                nc.vector.tensor_mul(ig[:, sl], i_t[:, sl], g_t[:, sl])
                if t == 0:
                    nc.vector.tensor_copy(c_st[:, sl], ig[:, sl])
                else:
                    nc.vector.tensor_mul(c_st[:, sl], f_t[:, sl], c_st[:, sl])
                    nc.vector.tensor_add(c_st[:, sl], c_st[:, sl], ig[:, sl])
                nc.scalar.activation(tc_t[:, sl], c_st[:, sl], ACTF.Tanh)
                nc.vector.tensor_mul(h_st[:, sl], o_t[:, sl], tc_t[:, sl])
                # transpose this half's 4 chunks right away
                for k in range(hh * 4, hh * 4 + 4):
                    tp = b_ps.tile([128, 64], F32, tag="ps")
                    nc.tensor.transpose(tp[:], h_st[:, k * 128:(k + 1) * 128],
                                        ident[:].bitcast(F32))
                    ht_k = b_hT.tile([128, 64], F32R, tag="hT",
                                     name=f"hT{k}_{t}")
                    nc.vector.tensor_copy(ht_k[:], tp[:])
                    nc.scalar.dma_start(
                        hsT_d[:, t * 512 + k * 64:t * 512 + (k + 1) * 64],
                        ht_k[:])
                    hh_done.append(ht_k)
            hT_prev = hh_done
        if rep_cm is not None:
            rep_cm.__exit__(None, None, None)

def _phase_c(tc, nc, wt_d, bl_d, hsT_d, out_d, repeat=1):
    # ---------------- Phase C: logits slice = hs @ WT + bl ---------------
    with tc.tile_pool(name="c_sb", bufs=1) as c_sb, \
         tc.tile_pool(name="c_hs", bufs=24) as c_hs, \
         tc.tile_pool(name="c_out", bufs=12) as c_out, \
         tc.tile_pool(name="c_ps", bufs=8, space="PSUM") as c_ps:
        wt_sb = [c_sb.tile([128, VS], F32R, tag=f"wt{k}", name=f"wtsb{k}") for k in range(HC)]
        for k in range(HC):
            nc.sync.dma_start(wt_sb[k][:], wt_d[k * 128:(k + 1) * 128, :])
        bl_sb = c_sb.tile([128, VS], F32, tag="bl")
        nc.sync.dma_start(bl_sb[:], bl_d[:, :])

        hsT4 = hsT_d.rearrange("p (t k b) -> p t k b", t=T, k=HC, b=64)
        rep_cm = _rep_loop(tc, nc, c_sb, repeat)
        if rep_cm is not None:
            rep_cm.__enter__()
        for m in range(MC):
            t0, t1 = 2 * m, 2 * m + 1
            hs_k = []
            for k in range(HC):
                hk = c_hs.tile([128, 128], F32R, tag="slab", name=f"hk{k}_{m}")
                nc.sync.dma_start(
                    hk[:].rearrange("p (t b) -> p t b", t=2, b=64),
                    hsT4[:, t0:t0 + 2, k, :])
                hs_k.append(hk)
            for n in range(NV):
                ps = c_ps.tile([128, VN], F32)
                for k in range(HC):
                    nc.tensor.matmul(
                        ps[:], hs_k[k][:],
                        wt_sb[k][:, n * VN:(n + 1) * VN],
                        start=(k == 0), stop=(k == HC - 1))
                o_sb = c_out.tile([128, VN], F32)
                nc.vector.tensor_add(o_sb[:], ps[:],
                                     bl_sb[:, n * VN:(n + 1) * VN])
                eng0 = (nc.sync, nc.scalar)[n % 2]
                eng1 = (nc.scalar, nc.sync)[n % 2]
                eng0.dma_start(out_d[:, t0, ds(n * VN, VN)], o_sb[0:64, :])
                eng1.dma_start(out_d[:, t1, ds(n * VN, VN)], o_sb[64:128, :])
        if rep_cm is not None:
            rep_cm.__exit__(None, None, None)


def build_program(phases=None, with_reps=False):
    import os
    if phases is None:
        phases = os.environ.get("KPHASES", "abc")
    nc = bacc.Bacc("TRN2", target_bir_lowering=False, debug=False,
                   num_devices=NCORES)
    io = {}
    if with_reps:
        io["reps"] = nc.dram_tensor("reps", [1, 1], mybir.dt.int32,
                                    kind="ExternalInput").ap()
    io |= {
        "xT": nc.dram_tensor("xT", [E, TOK], F32R, kind="ExternalInput").ap(),
        "w": nc.dram_tensor("w", [E, G4], F32R, kind="ExternalInput").ap(),
        "u": nc.dram_tensor("u", [H, G4], F32R, kind="ExternalInput").ap(),
        "bg": nc.dram_tensor("bg", [128, G4], F32, kind="ExternalInput").ap(),
        "ident": nc.dram_tensor("ident", [64, 64], F32R,
                                kind="ExternalInput").ap(),
        "wt": nc.dram_tensor("wt", [H, VS], F32R, kind="ExternalInput").ap(),
        "bl": nc.dram_tensor("bl", [128, VS], F32, kind="ExternalInput").ap(),
        "xg_scratch": nc.dram_tensor("xg_scratch", [TOK, G4], F32R,
                                     kind="Internal").ap(),
        "hsT_scratch": nc.dram_tensor("hsT_scratch", [128, T * 512], F32R,
                                      kind="Internal").ap(),
        "logits": nc.dram_tensor("logits", [B, T, VS], F32,
                                 kind="ExternalOutput").ap(),
    }
    with tile.TileContext(nc) as tc:
        with ExitStack() as ctx:
            _emit(ctx, tc, io, phases)
    nc.compile()
    return nc


def make_in_maps(features, captions, embedding, W_i, U_i, b_i, W_f, U_f, b_f,
                 W_g, U_g, b_g, W_o, U_o, b_o, linear_w, linear_b):
    features = np.asarray(features, dtype=np.float32)
    captions = np.asarray(captions)
    embedding = np.asarray(embedding, dtype=np.float32)
    emb = embedding[captions[:, :-1]]                        # [B, T-1, E]
    x_seq = np.concatenate([features[:, None, :], emb], axis=1)  # [B, T, E]
    x_flat = np.ascontiguousarray(
        x_seq.transpose(1, 0, 2).reshape(TOK, E))            # t-major tokens
    xT = _round_tf32(np.ascontiguousarray(x_flat.T))         # [E, TOK]

    w = _round_tf32(np.concatenate([W_i, W_f, W_g, W_o], axis=1))  # [E, 4H]
    u = _round_tf32(np.concatenate([U_i, U_f, U_g, U_o], axis=1))  # [H, 4H]
    bgv = np.concatenate([b_i, b_f, b_g, b_o], axis=0).astype(np.float32)
    bg = np.ascontiguousarray(np.broadcast_to(bgv[None, :], (128, G4)))
    ident = np.eye(64, dtype=np.float32)

    linear_w = np.asarray(linear_w, dtype=np.float32)
    linear_b = np.asarray(linear_b, dtype=np.float32)
    common = {"xT": xT, "w": w, "u": u, "bg": bg, "ident": ident}
    in_maps = []
    for c in range(NCORES):
        wt = _round_tf32(
            np.ascontiguousarray(linear_w[c * VS:(c + 1) * VS, :].T))
        bl = np.ascontiguousarray(np.broadcast_to(
            linear_b[None, c * VS:(c + 1) * VS], (128, VS)))
        in_maps.append({**common, "wt": wt, "bl": bl})
    return in_maps


_PROGRAM = None


def kernel(**inputs) -> np.ndarray:
    global _PROGRAM
    if _PROGRAM is None:
        _PROGRAM = build_program()
    in_maps = make_in_maps(**inputs)
    res = bass_utils.run_bass_kernel_spmd(
        _PROGRAM, in_maps, core_ids=list(range(NCORES)))
    out = np.empty((B, T, V), dtype=np.float32)
    for c in range(NCORES):
        out[:, :, c * VS:(c + 1) * VS] = res.results[c]["logits"]
    return out



# revision 5
# speedup vs baseline: 1.4798x; 1.4798x over previous
"""Trainium2 Bass kernel for an LSTM caption decoder (DecoderRNN).

Math (fp32 reference):
  x_seq = [features; embedding[captions[:, :-1]]]      # [B, T, E]
  xg    = x_seq @ W + b                                # [T*B, 4H] (t-major)
  per step t: gates = xg_t + h @ U ; i,f,o=sigmoid, g=tanh
              c = f*c + i*g ; h = o*tanh(c)
  logits = hs @ linear_w.T + linear_b                  # [B, T, V]

B=64, T=64, E=512, H=1024, V=32000.

Distribution over 8 NeuronCores: the serial recurrence (and the xg GEMM
feeding it) is replicated on every core; the vocab projection (84% of
total FLOPs) is sharded column-wise: core c computes logits[:, :, c*4000:
(c+1)*4000]. No collectives; the host concatenates the vocab slices.

Single fused program per core:
  * Prologue: xg m-tiles (128 tokens x 4096 gate cols) in bf16, written
    to DRAM in a per-step "folded" layout [T][128=(Hhalf,batch), 2048].
  * Main loop over t: gates live in 4 PSUM tiles [128, 512] using the
    folded layout (partition = 64*s + b, col = h-dim within half s).
    xg is injected with one K=128 identity matmul per gate; the h@U
    matmuls accumulate on top in column-tiled pairs (tile_position
    (0,0) / (0,64)) so two M=64 matmuls run concurrently in the PE
    array. h is transposed on the PE into ring tiles that serve as
    stationaries for both the next step's U-matmuls and the vocab
    projection; vocab-projection matmuls for the previous 2 steps are
    interleaved into the PE stream to fill recurrence stalls.
All matmul operands are bf16 (fp32 PSUM accumulate, fp32 cell state).
Logits are stored bf16; the host upcasts and adds linear_b.
"""

from contextlib import ExitStack

import numpy as np
import ml_dtypes

import concourse.bass as bass
import concourse.mybir as mybir
import concourse.tile as tile
from concourse import bacc, bass_utils
from concourse.bass import ds

F32 = mybir.dt.float32
BF16 = mybir.dt.bfloat16
ACTF = mybir.ActivationFunctionType

B, T, E, H, V = 64, 64, 512, 1024, 32000
NCORES = 8
VS = V // NCORES          # vocab slice per core (4000)
G4 = 4 * H                # 4096
TOK = T * B               # 4096 tokens, t-major (row = t*B + b)
EC = E // 128             # 4  E chunks
HC = H // 128             # 8  H chunks
MC = TOK // 128           # 32 token m-tiles (2 timesteps each)
NV = 8                    # vocab N-chunks per core
VN = VS // NV             # 500 columns per vocab N-chunk

# gate order in the 4096 gate columns: i, f, g, o (blocks of 1024)
GI, GF, GG, GO = 0, 1, 2, 3
SIGM, TANH = None, None  # set lazily (ACTF enum)


def _bf16(x: np.ndarray) -> np.ndarray:
    return np.ascontiguousarray(
        np.asarray(x, dtype=np.float32).astype(ml_dtypes.bfloat16))


def _rep_loop(tc, nc, pool, repeat):
    """Repeat-loop context for timing (reps input) or None for repeat=1."""
    if isinstance(repeat, bass.AP):
        rt = pool.tile([1, 1], mybir.dt.int32, tag="reps", name="rt")
        nc.sync.dma_start(rt[:], repeat[:, :])
        with tc.tile_critical():
            tmp = nc.alloc_registers(f"reps_regs_{nc.next_id()}")
            nc.regs_load(tmp, rt[0:1, 0:1])
            n_reps = nc.snap(tmp, donate=True, min_val=1, max_val=1024)
        return tc.For_i(0, n_reps, 1)
    return tc.For_i(0, repeat, 1) if repeat > 1 else None


def _emit(tc, io):
    nc = tc.nc
    xT_d, w_d, u_d, bg_d, ident_d, wt_d = (
        io["xT"], io["w"], io["u"], io["bg"], io["ident"], io["wt"])
    xg_d, out_d = io["xg_scratch"], io["logits"]
    reps = io.get("reps", 1)

    # xg DRAM folded view: [T, 128, 2048] (partition = 64*s + b)
    xg3 = xg_d.rearrange("(t p) c -> t p c", t=T, p=128)

    with tc.tile_pool(name="wpool", bufs=1) as wpool, \
         tc.tile_pool(name="big", bufs=4) as bigp, \
         tc.tile_pool(name="xtp", bufs=2) as xtp, \
         tc.tile_pool(name="gate", bufs=5) as gatep, \
         tc.tile_pool(name="tmp", bufs=4) as tmpp, \
         tc.tile_pool(name="ring", bufs=2) as ringp, \
         tc.tile_pool(name="stg", bufs=6) as stgp, \
         tc.tile_pool(name="gps", bufs=4, space="PSUM") as gpsp, \
         tc.tile_pool(name="cps", bufs=2, space="PSUM") as cpsp, \
         tc.tile_pool(name="tps", bufs=2, space="PSUM") as tpsp:

        # ---- persistent weights (loaded once, reused across reps) ----
        u_sb = [wpool.tile([128, G4], BF16, tag=f"u{k}", name=f"usb{k}")
                for k in range(HC)]
        for k in range(HC):
            nc.sync.dma_start(u_sb[k][:], u_d[k * 128:(k + 1) * 128, :])
        wt_sb = [wpool.tile([128, VS], BF16, tag=f"wt{k}", name=f"wtsb{k}")
                 for k in range(HC)]
        for k in range(HC):
            nc.sync.dma_start(wt_sb[k][:], wt_d[k * 128:(k + 1) * 128, :])
        bg_sb = wpool.tile([128, G4], BF16, tag="bg")
        nc.sync.dma_start(bg_sb[:], bg_d[:, :])
        ident = wpool.tile([128, 128], BF16, tag="ident")
        nc.sync.dma_start(ident[:], ident_d[:, :])
        # persistent fp32 state (re-initialized at t=0 of every rep)
        c_st = wpool.tile([128, 512], F32, tag="c")
        h_bf = wpool.tile([128, 512], BF16, tag="h")

        rep_cm = _rep_loop(tc, nc, wpool, reps)
        if rep_cm is not None:
            rep_cm.__enter__()

        # ================= Prologue: xg = x @ W + b =================
        # w chunks share the "big" tag with the main loop's xg tiles so
        # the SBUF slots are reused once the prologue is done with them.
        w_sb = [bigp.tile([128, G4], BF16, tag="big", name=f"wsb{k}")
                for k in range(EC)]
        for k in range(EC):
            nc.sync.dma_start(w_sb[k][:], w_d[k * 128:(k + 1) * 128, :])
        for m in range(MC):
            xt_m = xtp.tile([128, 4 * 128], BF16, tag="xt", name=f"xt{m}")
            for k in range(EC):
                nc.sync.dma_start(
                    xt_m[:, k * 128:(k + 1) * 128],
                    xT_d[k * 128:(k + 1) * 128, m * 128:(m + 1) * 128])
            for q in range(8):   # q = 2*g + s
                ps = cpsp.tile([128, 512], F32, tag="cps", name=f"aps{m}_{q}")
                for k in range(EC):
                    nc.tensor.matmul(
                        ps[:], xt_m[:, k * 128:(k + 1) * 128],
                        w_sb[k][:, q * 512:(q + 1) * 512],
                        start=(k == 0), stop=(k == EC - 1))
                st = stgp.tile([128, 512], BF16, tag="stg", name=f"ast{m}_{q}")
                eng = (nc.vector.tensor_add, nc.vector.tensor_add)[q % 2]
                eng(st[:], ps[:], bg_sb[:, q * 512:(q + 1) * 512])
                g_, s_ = q // 2, q % 2
                for p in range(2):
                    nc.scalar.dma_start(
                        xg3[2 * m + p, 64 * s_:64 * s_ + 64,
                            512 * g_:512 * g_ + 512],
                        st[64 * p:64 * p + 64, :])

        # ================= Main loop =================
        # ring tiles: R[k][:, 0:64] = hT chunk k at even step, [:, 64:128]
        # at odd step -> C stationary [128 h, 128 tokens] per m-tile.
        GORDER = (GI, GG, GF, GO)
        ring_prev = None          # [8] ring tiles of previous m-tile
        ring_cur = None           # [8] ring tiles of current m-tile
        hT_src = None             # (tiles, col_off) for last step's hT
        cwork = []                # deferred phase-C matmul thunks

        def _mk_thunks(rtiles, m):
            """Per-n-chunk thunks emitting the logits matmuls of m-tile m."""
            out = []
            for n in range(NV):
                def run(n=n, rtiles=rtiles, m=m):
                    ps = cpsp.tile([128, 512], F32, tag="cps",
                                   name=f"cps{m}_{n}")
                    for k in range(HC):
                        nc.tensor.matmul(
                            ps[:, 0:VN], rtiles[k][:],
                            wt_sb[k][:, n * VN:(n + 1) * VN],
                            start=(k == 0), stop=(k == HC - 1))
                    st = stgp.tile([128, 512], BF16, tag="stg",
                                   name=f"cst{m}_{n}")
                    cp = (nc.vector.tensor_copy, nc.scalar.copy)[n % 2]
                    cp(st[:, 0:VN], ps[:, 0:VN])
                    for p in range(2):
                        nc.sync.dma_start(
                            out_d[:, 2 * m + p, ds(n * VN, VN)],
                            st[64 * p:64 * p + 64, 0:VN])
                out.append(run)
            return out

        xg_tiles = {}

        def prefetch_xg(t):
            if t >= T:
                return
            xt = bigp.tile([128, 2048], BF16, tag="big", name=f"xg{t}")
            nc.sync.dma_start(xt[:], xg3[t, :, :])
            xg_tiles[t] = xt

        for t in range(3):
            prefetch_xg(t)

        for t in range(T):
            p = t % 2
            m = t // 2
            xg_t = xg_tiles.pop(t)

            # ---- gate PSUM tiles; identity matmuls inject xg ----
            # per-half injection keeps PSUM accumulation groups consistent
            # with the per-half U chains; (0,0)/(64,64) tiles run concurrent.
            ps_g = {}
            for g_ in GORDER:
                ps = gpsp.tile([128, 512], F32, tag="gps",
                               name=f"gps{g_}_{t}")
                for s_ in range(2):
                    o_ = 64 * s_
                    nc.tensor.matmul(ps[o_:o_ + 64, :],
                                     ident[o_:o_ + 64, o_:o_ + 64],
                                     xg_t[o_:o_ + 64,
                                          512 * g_:512 * g_ + 512],
                                     start=True, stop=(t == 0),
                                     skip_group_check=True)
                ps_g[g_] = ps

            # ---- h @ U accumulation, col-tiled pairs ----
            if t > 0:
                src, off = hT_src
                for g_ in GORDER:
                    for k in range(HC):
                        lhsT = src[k][:, off:off + 64]
                        for s_ in range(2):
                            ucol = g_ * 1024 + 512 * s_
                            nc.tensor.matmul(
                                ps_g[g_][64 * s_:64 * s_ + 64, :],
                                lhsT, u_sb[k][:, ucol:ucol + 512],
                                start=False, stop=(k == HC - 1),
                                skip_group_check=True)

            # ---- activations + elementwise (folded [128,512]) ----
            i_t = gatep.tile([128, 512], F32, tag="gate", name=f"gi{t}")
            nc.scalar.activation(i_t[:], ps_g[GI][:], ACTF.Sigmoid)
            g_t = gatep.tile([128, 512], F32, tag="gate", name=f"gg{t}")
            nc.scalar.activation(g_t[:], ps_g[GG][:], ACTF.Tanh)
            ig = tmpp.tile([128, 512], F32, tag="tmp", name=f"ig{t}")
            nc.vector.tensor_mul(ig[:], i_t[:], g_t[:])
            f_t = gatep.tile([128, 512], F32, tag="gate", name=f"gf{t}")
            nc.scalar.activation(f_t[:], ps_g[GF][:], ACTF.Sigmoid)
            if t == 0:
                nc.vector.tensor_copy(c_st[:], ig[:])
            else:
                nc.vector.tensor_mul(c_st[:], f_t[:], c_st[:])
                nc.vector.tensor_add(c_st[:], c_st[:], ig[:])
            tc_t = tmpp.tile([128, 512], F32, tag="tmp", name=f"tc{t}")
            nc.scalar.activation(tc_t[:], c_st[:], ACTF.Tanh)
            o_t = gatep.tile([128, 512], F32, tag="gate", name=f"go{t}")
            nc.scalar.activation(o_t[:], ps_g[GO][:], ACTF.Sigmoid)
            nc.vector.tensor_mul(h_bf[:], o_t[:], tc_t[:])

            # ---- phase-C interleave (first slice) then transposes ----
            burst1 = min(len(cwork), 2)
            for _ in range(burst1):
                cwork.pop(0)()

            # transpose h into ring tiles for this m-tile
            if p == 0:
                ring_cur = [ringp.tile([128, 128], BF16, tag=f"ring{k}",
                                       name=f"ring{k}_{m}")
                            for k in range(HC)]
            tp = tpsp.tile([128, 512], BF16, tag="tps", name=f"tps{t}")
            for j in range(4):
                nc.tensor.transpose(tp[:, 128 * j:128 * j + 128],
                                    h_bf[:, 128 * j:128 * j + 128],
                                    ident[:])
            for j in range(4):
                cp = (nc.vector.tensor_copy, nc.scalar.copy)[j % 2]
                cp(ring_cur[j][:, 64 * p:64 * p + 64],
                   tp[:, 128 * j:128 * j + 64])
                cp2 = (nc.scalar.copy, nc.vector.tensor_copy)[j % 2]
                cp2(ring_cur[j + 4][:, 64 * p:64 * p + 64],
                    tp[:, 128 * j + 64:128 * j + 128])
            hT_src = (ring_cur, 64 * p)

            # ---- remaining phase-C matmuls for this step ----
            burst2 = min(len(cwork), 4 - burst1) if t < T - 1 else len(cwork)
            for _ in range(burst2):
                cwork.pop(0)()

            prefetch_xg(t + 3)
            if p == 1:
                cwork.extend(_mk_thunks(ring_cur, m))
                ring_prev = ring_cur

        # epilogue: remaining C work (last m-tile)
        for thunk in cwork:
            thunk()

        if rep_cm is not None:
            rep_cm.__exit__(None, None, None)


def build_program(with_reps=False):
    nc = bacc.Bacc("TRN2", target_bir_lowering=False, debug=False,
                   num_devices=NCORES)
    io = {}
    if with_reps:
        io["reps"] = nc.dram_tensor("reps", [1, 1], mybir.dt.int32,
                                    kind="ExternalInput").ap()
    io |= {
        "xT": nc.dram_tensor("xT", [E, TOK], BF16, kind="ExternalInput").ap(),
        "w": nc.dram_tensor("w", [E, G4], BF16, kind="ExternalInput").ap(),
        "u": nc.dram_tensor("u", [H, G4], BF16, kind="ExternalInput").ap(),
        "bg": nc.dram_tensor("bg", [128, G4], BF16,
                             kind="ExternalInput").ap(),
        "ident": nc.dram_tensor("ident", [128, 128], BF16,
                                kind="ExternalInput").ap(),
        "wt": nc.dram_tensor("wt", [H, VS], BF16, kind="ExternalInput").ap(),
        "xg_scratch": nc.dram_tensor("xg_scratch", [T * 128, 2048], BF16,
                                     kind="Internal").ap(),
        "logits": nc.dram_tensor("logits", [B, T, VS], BF16,
                                 kind="ExternalOutput").ap(),
    }
    with tile.TileContext(nc) as tc:
        _emit(tc, io)
    nc.compile()
    return nc


def make_in_maps(features, captions, embedding, W_i, U_i, b_i, W_f, U_f, b_f,
                 W_g, U_g, b_g, W_o, U_o, b_o, linear_w, linear_b):
    features = np.asarray(features, dtype=np.float32)
    captions = np.asarray(captions)
    embedding = np.asarray(embedding, dtype=np.float32)
    emb = embedding[captions[:, :-1]]                        # [B, T-1, E]
    x_seq = np.concatenate([features[:, None, :], emb], axis=1)  # [B, T, E]
    x_flat = np.ascontiguousarray(
        x_seq.transpose(1, 0, 2).reshape(TOK, E))            # t-major tokens
    xT = _bf16(x_flat.T)                                     # [E, TOK]

    w = _bf16(np.concatenate([W_i, W_f, W_g, W_o], axis=1))  # [E, 4H]
    u = _bf16(np.concatenate([U_i, U_f, U_g, U_o], axis=1))  # [H, 4H]
    bgv = np.concatenate([b_i, b_f, b_g, b_o], axis=0).astype(np.float32)
    bg = _bf16(np.broadcast_to(bgv[None, :], (128, G4)))
    ident = _bf16(np.eye(128, dtype=np.float32))

    linear_w = np.asarray(linear_w, dtype=np.float32)
    common = {"xT": xT, "w": w, "u": u, "bg": bg, "ident": ident}
    in_maps = []
    for c in range(NCORES):
        wt = _bf16(linear_w[c * VS:(c + 1) * VS, :].T)
        in_maps.append({**common, "wt": wt})
    return in_maps


_PROGRAM = None


def kernel(**inputs) -> np.ndarray:
    global _PROGRAM
    if _PROGRAM is None:
        _PROGRAM = build_program()
    in_maps = make_in_maps(**inputs)
    res = bass_utils.run_bass_kernel_spmd(
        _PROGRAM, in_maps, core_ids=list(range(NCORES)))
    out = np.empty((B, T, V), dtype=np.float32)
    for c in range(NCORES):
        out[:, :, c * VS:(c + 1) * VS] = np.asarray(
            res.results[c]["logits"]).astype(np.float32)
    out += np.asarray(inputs["linear_b"], dtype=np.float32)[None, None, :]
    return out


# revision 7
# speedup vs baseline: 1.4855x; 1.0039x over previous
"""Trainium2 Bass kernel for an LSTM caption decoder (DecoderRNN).

Math (fp32 reference):
  x_seq = [features; embedding[captions[:, :-1]]]      # [B, T, E]
  xg    = x_seq @ W + b                                # [T*B, 4H] (t-major)
  per step t: gates = xg_t + h @ U ; i,f,o=sigmoid, g=tanh
              c = f*c + i*g ; h = o*tanh(c)
  logits = hs @ linear_w.T + linear_b                  # [B, T, V]

B=64, T=64, E=512, H=1024, V=32000.

Distribution over 8 NeuronCores: the serial recurrence (and the xg GEMM
feeding it) is replicated on every core; the vocab projection (84% of
total FLOPs) is sharded column-wise: core c computes logits[:, :, c*4000:
(c+1)*4000]. No collectives; the host concatenates the vocab slices.

Single fused program per core:
  * Prologue: xg m-tiles (128 tokens x 4096 gate cols) in bf16, written
    to DRAM in a per-step "folded" layout [T][128=(Hhalf,batch), 2048].
  * Main loop over t: gates live in 4 PSUM tiles [128, 512] using the
    folded layout (partition = 64*s + b, col = h-dim within half s).
    xg is injected with one K=128 identity matmul per gate; the h@U
    matmuls accumulate on top in column-tiled pairs (tile_position
    (0,0) / (0,64)) so two M=64 matmuls run concurrently in the PE
    array. h is transposed on the PE into ring tiles that serve as
    stationaries for both the next step's U-matmuls and the vocab
    projection; vocab-projection matmuls for the previous 2 steps are
    interleaved into the PE stream to fill recurrence stalls.
All matmul operands are bf16 (fp32 PSUM accumulate, fp32 cell state).
Logits are stored bf16; the host upcasts and adds linear_b.
"""

from contextlib import ExitStack

import numpy as np
import ml_dtypes

import concourse.bass as bass
import concourse.mybir as mybir
import concourse.tile as tile
from concourse import bacc, bass_utils
from concourse.bass import ds

F32 = mybir.dt.float32
BF16 = mybir.dt.bfloat16
ACTF = mybir.ActivationFunctionType

B, T, E, H, V = 64, 64, 512, 1024, 32000
NCORES = 8
VS = V // NCORES          # vocab slice per core (4000)
G4 = 4 * H                # 4096
TOK = T * B               # 4096 tokens, t-major (row = t*B + b)
EC = E // 128             # 4  E chunks
HC = H // 128             # 8  H chunks
MC = TOK // 128           # 32 token m-tiles (2 timesteps each)
NV = 8                    # vocab N-chunks per core
VN = VS // NV             # 500 columns per vocab N-chunk

# gate order in the 4096 gate columns: i, f, g, o (blocks of 1024)
GI, GF, GG, GO = 0, 1, 2, 3
SIGM, TANH = None, None  # set lazily (ACTF enum)


def _bf16(x: np.ndarray) -> np.ndarray:
    return np.ascontiguousarray(
        np.asarray(x, dtype=np.float32).astype(ml_dtypes.bfloat16))


def _rep_loop(tc, nc, pool, repeat):
    """Repeat-loop context for timing (reps input) or None for repeat=1."""
    if isinstance(repeat, bass.AP):
        rt = pool.tile([1, 1], mybir.dt.int32, tag="reps", name="rt")
        nc.sync.dma_start(rt[:], repeat[:, :])
        with tc.tile_critical():
            tmp = nc.alloc_registers(f"reps_regs_{nc.next_id()}")
            nc.regs_load(tmp, rt[0:1, 0:1])
            n_reps = nc.snap(tmp, donate=True, min_val=1, max_val=1024)
        return tc.For_i(0, n_reps, 1)
    return tc.For_i(0, repeat, 1) if repeat > 1 else None


def _emit(tc, io):
    nc = tc.nc
    xT_d, w_d, u_d, bg_d, ident_d, wt_d = (
        io["xT"], io["w"], io["u"], io["bg"], io["ident"], io["wt"])
    xg_d, out_d = io["xg_scratch"], io["logits"]
    reps = io.get("reps", 1)

    # xg DRAM folded view: [T, 128, 2048] (partition = 64*s + b)
    xg3 = xg_d.rearrange("(t p) c -> t p c", t=T, p=128)

    with tc.tile_pool(name="wpool", bufs=1) as wpool, \
         tc.tile_pool(name="big", bufs=4) as bigp, \
         tc.tile_pool(name="xtp", bufs=2) as xtp, \
         tc.tile_pool(name="gate", bufs=5) as gatep, \
         tc.tile_pool(name="tmp", bufs=4) as tmpp, \
         tc.tile_pool(name="ring", bufs=2) as ringp, \
         tc.tile_pool(name="stg", bufs=6) as stgp, \
         tc.tile_pool(name="gps", bufs=4, space="PSUM") as gpsp, \
         tc.tile_pool(name="cps", bufs=3, space="PSUM") as cpsp, \
         tc.tile_pool(name="tps", bufs=1, space="PSUM") as tpsp:

        # ---- persistent weights (loaded once, reused across reps) ----
        u_sb = [wpool.tile([128, G4], BF16, tag=f"u{k}", name=f"usb{k}")
                for k in range(HC)]
        for k in range(HC):
            nc.sync.dma_start(u_sb[k][:], u_d[k * 128:(k + 1) * 128, :])
        wt_sb = [wpool.tile([128, VS], BF16, tag=f"wt{k}", name=f"wtsb{k}")
                 for k in range(HC)]
        for k in range(HC):
            nc.sync.dma_start(wt_sb[k][:], wt_d[k * 128:(k + 1) * 128, :])
        bg_sb = wpool.tile([128, G4], BF16, tag="bg")
        nc.sync.dma_start(bg_sb[:], bg_d[:, :])
        ident = wpool.tile([128, 128], BF16, tag="ident")
        nc.sync.dma_start(ident[:], ident_d[:, :])
        # persistent fp32 state (re-initialized at t=0 of every rep)
        c_st = wpool.tile([128, 512], F32, tag="c")
        h_bf = wpool.tile([128, 512], BF16, tag="h")

        rep_cm = _rep_loop(tc, nc, wpool, reps)
        if rep_cm is not None:
            rep_cm.__enter__()

        # ================= Prologue: xg = x @ W + b =================
        # w chunks share the "big" tag with the main loop's xg tiles so
        # the SBUF slots are reused once the prologue is done with them.
        w_sb = [bigp.tile([128, G4], BF16, tag="big", name=f"wsb{k}")
                for k in range(EC)]
        for k in range(EC):
            nc.sync.dma_start(w_sb[k][:], w_d[k * 128:(k + 1) * 128, :])
        for m in range(MC):
            xt_m = xtp.tile([128, 4 * 128], BF16, tag="xt", name=f"xt{m}")
            for k in range(EC):
                nc.sync.dma_start(
                    xt_m[:, k * 128:(k + 1) * 128],
                    xT_d[k * 128:(k + 1) * 128, m * 128:(m + 1) * 128])
            for q in range(8):   # q = 2*g + s
                ps = cpsp.tile([128, 512], F32, tag="cps", name=f"aps{m}_{q}")
                for k in range(EC):
                    nc.tensor.matmul(
                        ps[:], xt_m[:, k * 128:(k + 1) * 128],
                        w_sb[k][:, q * 512:(q + 1) * 512],
                        start=(k == 0), stop=(k == EC - 1))
                st = stgp.tile([128, 512], BF16, tag="stg", name=f"ast{m}_{q}")
                eng = (nc.vector.tensor_add, nc.vector.tensor_add)[q % 2]
                eng(st[:], ps[:], bg_sb[:, q * 512:(q + 1) * 512])
                g_, s_ = q // 2, q % 2
                for p in range(2):
                    nc.scalar.dma_start(
                        xg3[2 * m + p, 64 * s_:64 * s_ + 64,
                            512 * g_:512 * g_ + 512],
                        st[64 * p:64 * p + 64, :])

        # ================= Main loop =================
        # ring tiles: R[k][:, 0:64] = hT chunk k at even step, [:, 64:128]
        # at odd step -> C stationary [128 h, 128 tokens] per m-tile.
        GORDER = (GI, GG, GF, GO)
        ring_prev = None          # [8] ring tiles of previous m-tile
        ring_cur = None           # [8] ring tiles of current m-tile
        hT_src = None             # (tiles, col_off) for last step's hT
        cwork = []                # deferred phase-C matmul thunks

        def _mk_thunks(rtiles, m):
            """Per-n-chunk thunks emitting the logits matmuls of m-tile m."""
            out = []
            for n in range(NV):
                def run(n=n, rtiles=rtiles, m=m):
                    ps = cpsp.tile([128, 512], F32, tag="cps",
                                   name=f"cps{m}_{n}")
                    for k in range(HC):
                        nc.tensor.matmul(
                            ps[:, 0:VN], rtiles[k][:],
                            wt_sb[k][:, n * VN:(n + 1) * VN],
                            start=(k == 0), stop=(k == HC - 1))
                    st = stgp.tile([128, 512], BF16, tag="stg",
                                   name=f"cst{m}_{n}")
                    cp = (nc.vector.tensor_copy, nc.scalar.copy)[n % 2]
                    cp(st[:, 0:VN], ps[:, 0:VN])
                    for p in range(2):
                        nc.sync.dma_start(
                            out_d[:, 2 * m + p, ds(n * VN, VN)],
                            st[64 * p:64 * p + 64, 0:VN])
                out.append(run)
            return out

        xg_tiles = {}

        def prefetch_xg(t):
            if t >= T:
                return
            xt = bigp.tile([128, 2048], BF16, tag="big", name=f"xg{t}")
            nc.sync.dma_start(xt[:], xg3[t, :, :])
            xg_tiles[t] = xt

        for t in range(3):
            prefetch_xg(t)

        for t in range(T):
            p = t % 2
            m = t // 2
            xg_t = xg_tiles.pop(t)

            # ---- gate PSUM tiles; identity matmuls inject xg ----
            # per-half injection keeps PSUM accumulation groups consistent
            # with the per-half U chains; (0,0)/(64,64) tiles run concurrent.
            ps_g = {}
            for g_ in GORDER:
                ps = gpsp.tile([128, 512], F32, tag="gps",
                               name=f"gps{g_}_{t}")
                for s_ in range(2):
                    o_ = 64 * s_
                    nc.tensor.matmul(ps[o_:o_ + 64, :],
                                     ident[o_:o_ + 64, o_:o_ + 64],
                                     xg_t[o_:o_ + 64,
                                          512 * g_:512 * g_ + 512],
                                     start=True, stop=(t == 0),
                                     skip_group_check=True)
                ps_g[g_] = ps

            # ---- h @ U accumulation, col-tiled pairs ----
            if t > 0:
                src, off = hT_src
                for g_ in GORDER:
                    for k in range(HC):
                        lhsT = src[k][:, off:off + 64]
                        for s_ in range(2):
                            ucol = g_ * 1024 + 512 * s_
                            nc.tensor.matmul(
                                ps_g[g_][64 * s_:64 * s_ + 64, :],
                                lhsT, u_sb[k][:, ucol:ucol + 512],
                                start=False, stop=(k == HC - 1),
                                skip_group_check=True)

            # ---- activations + elementwise (folded [128,512]) ----
            i_t = gatep.tile([128, 512], F32, tag="gate", name=f"gi{t}")
            nc.scalar.activation(i_t[:], ps_g[GI][:], ACTF.Sigmoid)
            g_t = gatep.tile([128, 512], F32, tag="gate", name=f"gg{t}")
            nc.scalar.activation(g_t[:], ps_g[GG][:], ACTF.Tanh)
            ig = tmpp.tile([128, 512], F32, tag="tmp", name=f"ig{t}")
            nc.vector.tensor_mul(ig[:], i_t[:], g_t[:])
            f_t = gatep.tile([128, 512], F32, tag="gate", name=f"gf{t}")
            nc.scalar.activation(f_t[:], ps_g[GF][:], ACTF.Sigmoid)
            if t == 0:
                nc.vector.tensor_copy(c_st[:], ig[:])
            else:
                nc.vector.tensor_mul(c_st[:], f_t[:], c_st[:])
                nc.vector.tensor_add(c_st[:], c_st[:], ig[:])
            o_t = gatep.tile([128, 512], F32, tag="gate", name=f"go{t}")
            nc.scalar.activation(o_t[:], ps_g[GO][:], ACTF.Sigmoid)
            tc_t = tmpp.tile([128, 512], F32, tag="tmp", name=f"tc{t}")
            nc.scalar.activation(tc_t[:], c_st[:], ACTF.Tanh)
            nc.vector.tensor_mul(h_bf[:], o_t[:], tc_t[:])

            # ---- phase-C interleave (first slice) then transposes ----
            burst1 = min(len(cwork), 2)
            for _ in range(burst1):
                cwork.pop(0)()

            # transpose h into ring tiles for this m-tile
            if p == 0:
                ring_cur = [ringp.tile([128, 128], BF16, tag=f"ring{k}",
                                       name=f"ring{k}_{m}")
                            for k in range(HC)]
            tp = tpsp.tile([128, 512], BF16, tag="tps", name=f"tps{t}")
            for j in range(4):
                nc.tensor.transpose(tp[:, 128 * j:128 * j + 128],
                                    h_bf[:, 128 * j:128 * j + 128],
                                    ident[:])
            for j in range(4):
                cp = (nc.vector.tensor_copy, nc.scalar.copy)[j % 2]
                cp(ring_cur[j][:, 64 * p:64 * p + 64],
                   tp[:, 128 * j:128 * j + 64])
                cp2 = (nc.scalar.copy, nc.vector.tensor_copy)[j % 2]
                cp2(ring_cur[j + 4][:, 64 * p:64 * p + 64],
                    tp[:, 128 * j + 64:128 * j + 128])
            hT_src = (ring_cur, 64 * p)

            # ---- remaining phase-C matmuls for this step ----
            burst2 = min(len(cwork), 4 - burst1) if t < T - 1 else len(cwork)
            for _ in range(burst2):
                cwork.pop(0)()

            prefetch_xg(t + 3)
            if p == 1:
                cwork.extend(_mk_thunks(ring_cur, m))
                ring_prev = ring_cur

        # epilogue: remaining C work (last m-tile)
        for thunk in cwork:
            thunk()

        if rep_cm is not None:
            rep_cm.__exit__(None, None, None)


def build_program(with_reps=False):
    nc = bacc.Bacc("TRN2", target_bir_lowering=False, debug=False,
                   num_devices=NCORES)
    io = {}
    if with_reps:
        io["reps"] = nc.dram_tensor("reps", [1, 1], mybir.dt.int32,
                                    kind="ExternalInput").ap()
    io |= {
        "xT": nc.dram_tensor("xT", [E, TOK], BF16, kind="ExternalInput").ap(),
        "w": nc.dram_tensor("w", [E, G4], BF16, kind="ExternalInput").ap(),
        "u": nc.dram_tensor("u", [H, G4], BF16, kind="ExternalInput").ap(),
        "bg": nc.dram_tensor("bg", [128, G4], BF16,
                             kind="ExternalInput").ap(),
        "ident": nc.dram_tensor("ident", [128, 128], BF16,
                                kind="ExternalInput").ap(),
        "wt": nc.dram_tensor("wt", [H, VS], BF16, kind="ExternalInput").ap(),
        "xg_scratch": nc.dram_tensor("xg_scratch", [T * 128, 2048], BF16,
                                     kind="Internal").ap(),
        "logits": nc.dram_tensor("logits", [B, T, VS], BF16,
                                 kind="ExternalOutput").ap(),
    }
    with tile.TileContext(nc) as tc:
        _emit(tc, io)
    nc.compile()
    return nc


def make_in_maps(features, captions, embedding, W_i, U_i, b_i, W_f, U_f, b_f,
                 W_g, U_g, b_g, W_o, U_o, b_o, linear_w, linear_b):
    features = np.asarray(features, dtype=np.float32)
    captions = np.asarray(captions)
    embedding = np.asarray(embedding, dtype=np.float32)
    emb = embedding[captions[:, :-1]]                        # [B, T-1, E]
    x_seq = np.concatenate([features[:, None, :], emb], axis=1)  # [B, T, E]
    x_flat = np.ascontiguousarray(
        x_seq.transpose(1, 0, 2).reshape(TOK, E))            # t-major tokens
    xT = _bf16(x_flat.T)                                     # [E, TOK]

    w = _bf16(np.concatenate([W_i, W_f, W_g, W_o], axis=1))  # [E, 4H]
    u = _bf16(np.concatenate([U_i, U_f, U_g, U_o], axis=1))  # [H, 4H]
    bgv = np.concatenate([b_i, b_f, b_g, b_o], axis=0).astype(np.float32)
    bg = _bf16(np.broadcast_to(bgv[None, :], (128, G4)))
    ident = _bf16(np.eye(128, dtype=np.float32))

    linear_w = np.asarray(linear_w, dtype=np.float32)
    common = {"xT": xT, "w": w, "u": u, "bg": bg, "ident": ident}
    in_maps = []
    for c in range(NCORES):
        wt = _bf16(linear_w[c * VS:(c + 1) * VS, :].T)
        in_maps.append({**common, "wt": wt})
    return in_maps


_PROGRAM = None


def kernel(**inputs) -> np.ndarray:
    global _PROGRAM
    if _PROGRAM is None:
        _PROGRAM = build_program()
    in_maps = make_in_maps(**inputs)
    res = bass_utils.run_bass_kernel_spmd(
        _PROGRAM, in_maps, core_ids=list(range(NCORES)))
    out = np.empty((B, T, V), dtype=np.float32)
    for c in range(NCORES):
        out[:, :, c * VS:(c + 1) * VS] = np.asarray(
            res.results[c]["logits"]).astype(np.float32)
    out += np.asarray(inputs["linear_b"], dtype=np.float32)[None, None, :]
    return out
